# revision 45
# baseline (speedup 1.0000x reference)
"""Memory-efficient Dice loss on 8 Trainium2 NeuronCores.

Full inputs:
  logits  (2, 16, 64, 128, 128) fp32
  targets (2, 64, 128, 128) int  (values 0..15)
Output: scalar fp32 loss = 1 - mean_{b, c != 0} dice[b, c].

Sharding: 8 cores over (B=2) x (D quartered into 4 slabs of 16).
Each core reduces its shard to a [128, 136] PSUM stats matrix; the host
combines the tiny per-core stats and applies the dice formula.

The dice statistics are permutation-invariant over voxels, and the host
controls the voxel -> (partition, position) layout completely. So the
host SORTS voxels by target class and deals them into fixed per-class
position ranges (identical across partitions/superblocks, padded with
dummy voxels to a compile-time quota). On device:

  e[j,c] = exp(logit[j,c])         (bf16, chunk-major, contiguous)
  Z[j]   = sum_c e[j,c]            (strided tensor_tensor tree, 2x mode)
  r[j]   = 1/Z[j]                  (fp32 approx reciprocal)
  mr[j,c'] = r[j] if j in class-c' range else 0
           = static-offset tensor_copy slices of r into a
             zero-initialized class-major R tile - no per-voxel masking
             compute at all; targets never reach the device.
  Stats via PSUM-accumulated bf16 matmuls contracting over partitions:
    weights = chunk-major e slice (cols g*16+c), moving = class-major
    [mr | r] window (cols c'*8+g'). Host extracts the g==g' entries.

Dummy voxels have all-zero logits: e = 1 (exact in bf16), Z = 16,
r = 1/16 (exact after bf16 rounding), so each dummy adds exactly 1/16
to probs_sum[c] for every c and to its range-class intersection; the
host knows every dummy count and subtracts these contributions.

Per-class voxel counts are np.bincount host-side. The per-class quotas
are data-dependent compile-time constants: the program is cached per
quota tuple and rebuilt if an unseen input distribution changes them.

Measured-on-HW design notes:
  - logits host-permuted to [P, blocks, BW, C]: every DMA is 128
    contiguous ~8KB descriptors; exp reads and writes contiguously
    (ACT drops ~5x off peak on strided writes, int64-target gathers
    would flood the DMA queues with 4-byte descriptors).
  - DVE: scalar_tensor_tensor runs 1x on TRN2 HW (cost model claims
    4x); tensor_tensor runs 2x and tensor_copy 4x for packed 2-byte
    SBUF operands - hence the copy-based mask construction.
  - matmuls in bf16 (1 cycle/row vs 4 for fp32).
"""

import numpy as np
import ml_dtypes

import concourse.bass as bass
import concourse.mybir as mybir
import concourse.tile as tile
from concourse import bacc
from concourse.bass_utils import run_bass_kernel_spmd

B, C, D, H, W = 2, 16, 64, 128, 128
P = 128               # SBUF partitions
NCORES = 8
DSH = D // 4          # d-planes per core
N = DSH * H * W       # voxels per core = 262144
NSUP = 4              # compute superblocks
KSUB = 4              # DMA blocks per superblock
NBLK = NSUP * KSUB
GQ = 8                # positions per matmul (weights = 16*GQ = 128 cols)
M17 = C + 1           # mr classes + r slot
NOUT = M17 * GQ       # 136 output cols
SLOTS = P * NSUP      # slot groups a class quota is spread over

SMOOTH = 1.0
IGNORE_INDEX = 0


def _plan(quotas):
    """Derive layout constants from per-class quotas (elems per slot)."""
    qsum = int(np.sum(quotas))
    # SBW must split into KSUB integer blocks and GQ-wide matmul windows
    align = np.lcm(GQ, KSUB)
    sbw = ((qsum + align - 1) // align) * align
    offs = np.concatenate([[0], np.cumsum(quotas)]).astype(int)
    return sbw, offs


def build(quotas, lbufs=4, ebufs=2, rbufs=1):
    """Build the SPMD single-core Bass program for given class quotas."""
    SBW, offs = _plan(quotas)
    BW = SBW // KSUB
    nmm = SBW // GQ

    fp32 = mybir.dt.float32
    bf16 = mybir.dt.bfloat16
    AL = mybir.AluOpType

    nc = bacc.Bacc("TRN2", target_bir_lowering=False, debug=False)
    logits_d = nc.dram_tensor(
        "logits", [P, NBLK * C * BW], bf16, kind="ExternalInput"
    )
    out_d = nc.dram_tensor("out", [P, NOUT], fp32, kind="ExternalOutput")

    src_log = logits_d.ap().rearrange("p (b x) -> b p x", b=NBLK)

    def body(tc, pools):
        lpool, epool, rpool, zpool, psump, fin = pools

        acc = psump.tile([P, NOUT], fp32)

        # pre-zero the R buffer (split across GPSIMD and DVE); each
        # superblock then overwrites exactly the same static ranges
        R_tiles = [
            rpool.tile([P, M17 * SBW], bf16, tag=f"R{i}", name=f"R{i}", bufs=1)
            for i in range(rbufs)
        ]
        for R in R_tiles:
            half = (M17 * SBW) // 2
            nc.gpsimd.memset(R[:, 0:half], 0.0)
            nc.vector.memset(R[:, half:], 0.0)

        for sup in range(NSUP):
            E = epool.tile([P, C * SBW], bf16, tag="E")
            R = R_tiles[sup % rbufs]
            E3 = E[:].rearrange("p (j c) -> p j c", c=C)
            R3 = R[:].rearrange("p (c j) -> p c j", c=M17)
            rr = R[:, C * SBW : M17 * SBW]

            # DMA/exp parts and compute segments (position ranges within
            # the superblock). The very first block is split so the first
            # exp starts half a block earlier; the very last block's
            # compute is split to halve the serial drain chain.
            dma_parts = []
            segs = []
            for k in range(KSUB):
                j0, j1 = k * BW, (k + 1) * BW
                if (sup == 0 and k == 0) or (
                    sup == NSUP - 1 and k == KSUB - 1
                ):
                    jm = j0 + BW // 2
                    dma_parts += [(k, j0, jm), (k, jm, j1)]
                else:
                    dma_parts += [(k, j0, j1)]
                if sup == NSUP - 1 and k == KSUB - 1:
                    jm = j0 + BW // 2
                    segs += [(k, j0, jm), (k, jm, j1)]
                else:
                    segs += [(k, j0, j1)]
            nseg = len(segs)

            def exp_part(k, lo, hi):
                blk = sup * KSUB + k
                Lb = lpool.tile([P, C * (hi - lo)], bf16, tag="L")
                nc.sync.dma_start(
                    Lb[:], src_log[blk, :, (lo - k * BW) * C : (hi - k * BW) * C]
                )
                nc.scalar.activation(
                    E[:, lo * C : hi * C],
                    Lb[:],
                    mybir.ActivationFunctionType.Exp,
                )

            def compute_seg(si, lo, hi):
                W = hi - lo
                # Z = sum_c e: strided tensor_tensor tree (2x mode on
                # DVE), trailing in-place levels
                z = zpool.tile([P, 8 * W], bf16, tag="z")
                zf = zpool.tile([P, W], fp32, tag="zf")
                rf = zpool.tile([P, W], fp32, tag="rf")
                Eb = E3[:, lo:hi, :]
                z8 = z[:, 0 : 8 * W].rearrange("p (j c) -> p j c", c=8)
                z4 = z[:, 0 : 4 * W].rearrange("p (j c) -> p j c", c=4)
                z2 = z[:, 0 : 2 * W].rearrange("p (j c) -> p j c", c=2)
                nc.vector.tensor_tensor(z8[:], Eb[:, :, 0:8], Eb[:, :, 8:16], AL.add)
                nc.vector.tensor_tensor(z4[:], z8[:, :, 0:4], z8[:, :, 4:8], AL.add)
                nc.vector.tensor_tensor(z2[:], z4[:, :, 0:2], z4[:, :, 2:4], AL.add)
                nc.vector.tensor_tensor(zf[:], z2[:, :, 0], z2[:, :, 1], AL.add)

                # r = 1/Z fp32, bf16 cast into the r-column; alternate
                # the cast between GPSIMD and DVE so ACT stays pure-exp
                # (the pacing engine)
                nc.vector.reciprocal_approx_fast(rf[:], zf[:])
                if si % 2 == 0:
                    nc.gpsimd.tensor_copy(rr[:, lo:hi], rf[:])
                else:
                    nc.vector.tensor_copy(rr[:, lo:hi], rf[:])

                # mr[c'] = r on class ranges clipped to this segment,
                # copied on otherwise-idle GPSIMD
                for cc in range(C):
                    clo = max(int(offs[cc]), lo)
                    chi = min(int(offs[cc + 1]), hi)
                    if chi > clo:
                        nc.gpsimd.tensor_copy(
                            R[:, cc * SBW + clo : cc * SBW + chi],
                            rr[:, clo:chi],
                        )

                # stats matmuls whose window ends in this segment:
                # weights = contiguous chunk-major E slice (cols g*16+c),
                # moving = class-major R window (cols c'*8+g)
                for m in range(nmm):
                    end = (m + 1) * GQ - 1
                    if not (lo <= end < hi):
                        continue
                    first = sup == 0 and m == 0
                    last = sup == NSUP - 1 and m == nmm - 1
                    nc.tensor.matmul(
                        acc[:],
                        E[:, m * GQ * C : (m + 1) * GQ * C],
                        R3[:, :, m * GQ : (m + 1) * GQ],
                        start=first,
                        stop=last,
                    )

            # interleave: issue each DMA/exp part, then any compute
            # segment fully covered by the exp'd prefix
            covered = 0
            done = 0
            for (k, lo, hi) in dma_parts:
                exp_part(k, lo, hi)
                covered = hi
                while done < nseg and segs[done][2] <= covered:
                    si = sup * KSUB + done
                    compute_seg(si, segs[done][1], segs[done][2])
                    done += 1

        outs = fin.tile([P, NOUT], fp32)
        nc.vector.tensor_copy(outs[:], acc[:])
        nc.sync.dma_start(out_d.ap(), outs[:])

    with tile.TileContext(nc) as tc:
        with (
            tc.tile_pool(name="lpool", bufs=lbufs) as lpool,
            tc.tile_pool(name="epool", bufs=ebufs) as epool,
            tc.tile_pool(name="rpool", bufs=rbufs) as rpool,
            tc.tile_pool(name="zpool", bufs=3) as zpool,
            tc.tile_pool(name="psum", bufs=1, space="PSUM") as psump,
            tc.tile_pool(name="fin", bufs=1) as fin,
        ):
            pools = (lpool, epool, rpool, zpool, psump, fin)
            body(tc, pools)
    nc.compile()
    return nc


_NC_CACHE = {}


def _get_nc(quotas):
    key = tuple(int(q) for q in quotas)
    if key not in _NC_CACHE:
        _NC_CACHE[key] = build(np.asarray(key))
    return _NC_CACHE[key]


def _prep_core(lgT, tg, quotas, SBW, offs):
    """Sort one core's voxels by class into the quota layout.

    lgT: [N, C] bf16 contiguous; tg: [N] int targets.
    Returns (device logits [P, NBLK*C*BW] bf16, dummies-per-class [C]).
    """
    order = np.argsort(tg, kind="stable")
    counts = np.bincount(tg, minlength=C)[:C]
    # slot index (p, s, j) -> voxel id or -1
    slot = np.full((P, NSUP, SBW), -1, dtype=np.int64)
    cum = 0
    for c in range(C):
        q = int(quotas[c])
        ids = order[cum : cum + counts[c]]
        cum += counts[c]
        cap = q * SLOTS
        pad = np.full(cap, -1, dtype=np.int64)
        pad[: counts[c]] = ids
        # row-major deal: row j spreads over all (s, p) groups
        arr = pad.reshape(q, NSUP, P).transpose(2, 1, 0)  # [P, NSUP, q]
        slot[:, :, offs[c] : offs[c] + q] = arr
    flat = slot.reshape(-1)
    safe = np.where(flat < 0, 0, flat)
    lg = lgT[safe]                      # [P*NSUP*SBW, C]
    lg[flat < 0] = 0.0                  # dummy voxels: all-zero logits
    lg = lg.reshape(P, NBLK * (SBW // KSUB) * C)
    dummies = np.asarray(quotas) * SLOTS - counts
    return lg, counts, dummies


def prepare(logits, targets):
    """Quotas + compiled program + per-core inputs for the full inputs."""
    logits = np.asarray(logits)
    targets = np.asarray(targets).astype(np.int64, copy=False)

    # per-core class counts decide the compile-time quotas
    tgs, counts_i = [], []
    for i in range(NCORES):
        b, q = divmod(i, 4)
        tg = np.ascontiguousarray(targets[b, q * DSH : (q + 1) * DSH]).reshape(N)
        tgs.append(tg)
        counts_i.append(np.bincount(tg, minlength=C)[:C])
    counts_i = np.stack(counts_i)                       # [NCORES, C]
    quotas = (counts_i.max(axis=0) + SLOTS - 1) // SLOTS  # [C]
    SBW, offs = _plan(quotas)
    nc = _get_nc(quotas)

    in_maps = []
    dummies_i = np.zeros((NCORES, C), np.int64)
    for i in range(NCORES):
        b, q = divmod(i, 4)
        lgT = (
            logits[b, :, q * DSH : (q + 1) * DSH]
            .reshape(C, N)
            .T.astype(ml_dtypes.bfloat16)
        )
        lg, _, dmy = _prep_core(lgT, tgs[i], quotas, SBW, offs)
        dummies_i[i] = dmy
        in_maps.append({"logits": lg})
    return nc, in_maps, quotas, SBW, counts_i, dummies_i


def kernel(logits, targets):
    nc, in_maps, quotas, SBW, counts_i, dummies_i = prepare(logits, targets)
    res = run_bass_kernel_spmd(nc, in_maps, list(range(NCORES))).results

    inter = np.zeros((B, C), np.float64)
    probs = np.zeros((B, C), np.float64)
    for i in range(NCORES):
        o = res[i]["out"].astype(np.float64).reshape(GQ, C, M17, GQ)
        g = np.arange(GQ)
        od = o[g, :, :, g]                              # [g, c, c']
        it = od[:, np.arange(C), np.arange(C)].sum(axis=0)
        pr = od[:, :, C].sum(axis=0)
        # dummy corrections: each dummy contributes exactly 1/16 to its
        # range-class intersection and to probs_sum of every class
        # (all-zero logits: e = 1, r = 1/16, bf16-exact)
        n_slots_pad = NSUP * SBW * P - int(np.sum(quotas)) * SLOTS
        n_dmy_total = int(dummies_i[i].sum())
        it -= dummies_i[i] / 16.0
        pr -= n_dmy_total / 16.0
        # alignment-pad slots beyond the last class range also carry
        # r = 1/16 into probs_sum via the r column
        pr -= n_slots_pad / 16.0
        inter[i // 4] += it
        probs[i // 4] += pr

    counts = np.zeros((B, C), np.float64)
    for i in range(NCORES):
        counts[i // 4] += counts_i[i]

    dice = (2.0 * inter + SMOOTH) / (probs + counts + SMOOTH)
    mask = np.ones(C)
    mask[IGNORE_INDEX] = 0.0
    mean_dice = (dice * mask[None, :]).sum() / (B * (C - 1))
    return np.float32(1.0 - mean_dice)
